# revision 22
# baseline (speedup 1.0000x reference)
"""Trainium2 Bass kernel for the binarized-conv bottleneck block.

Math: out = prelu(prelu(bn3(bconv3(s3))) + x), where
  s1 = binarize(x); c1 = bconv1(s1) (1x1, 128->32)
  s2 = binarize(bn1(c1))  (prelu dropped: it preserves sign)
  c2 = bconv2(s2) (3x3 pad 1, 32->32); s3 = binarize(bn2(c2))
  c3 = bconv3(s3) (1x1, 32->128)

Key choices:
- Stage-1 binarization carried as g in {0,1} (weights 2*sign(w1), rowsum
  folded into the threshold); stages 2/3 carried as s in {-1,+1} via the
  scalar engine's Sign activation (weights sign(w), zero padding).  The
  binary path is exact; only the residual rounds (bf16), ~0.2% rel err
  vs the 2e-2 gate.
- x and out travel through HBM as bf16 (host casts): 8.4+8.4 MB per
  core, the DMA floor (~47us at 358 GB/s).
- 4 row-bands of 32, one per 32-partition group; bands 1,3 stored
  vertically flipped (host permute) so all groups process rows top-down
  in lockstep (single full-width instruction per elementwise stage; the
  3x3 taps of flipped groups use ky-reversed weight blocks).  This also
  lets conv2 run as ONE full-array matmul per tap with block-diagonal
  [128,128] fp8 weights (4 bands at once) - 18 matmuls per block
  instead of 72, which is what gets PE off the critical path.
- Work is blocked 4 rows (4096 px) at a time.  Engine split measured on
  HW: V: binarize(x), residual add, final prelu via TS(scale)+TT(max)
  (the fused STT runs at 0.5 elem/cyc - avoid); S: both Sign thresholds
  + the bn3-prelu (Sign and Prelu share every ACT table set, so no
  table reloads); gpsimd unused (bf16 elementwise is ~60us/op there).
- conv3 runs in quarter-tiles split across two pipeline iterations so
  PSUM double-buffering never stalls the in-order PE queue.

Sharding: data-parallel over batch, one image per NeuronCore (8 cores).
"""
import numpy as np
import ml_dtypes

import concourse.bass as bass
import concourse.mybir as mybir
from concourse import bacc
from concourse.tile import TileContext
from concourse.bass_utils import run_bass_kernel_spmd
F32 = mybir.dt.float32
BF16 = mybir.dt.bfloat16
FP8 = mybir.dt.float8e4
AF = mybir.ActivationFunctionType
OP = mybir.AluOpType

B, C, CI, H, W = 8, 128, 32, 128, 256
HW = H * W                    # 32768
BH = 32                       # band height (rows per band)
WP = W + 2                    # 258
SLAB = BH + 2                 # 34 rows: halo + 32 interior + halo
BLK = 4096                    # pixels per block = 4 rows x 4 bands
NBLK = 8
EPS = 1e-5

_CACHE = {}

# engine assignment: vector|gpsimd (SBUF-only ops)
CFG = dict(g1="vector", scale="vector", radd="vector", tmax="vector",
           wdt="bf16", w2dt="fp8")


def _eng(nc, name):
    return {"vector": nc.vector, "gpsimd": nc.gpsimd}[CFG[name]]


def _build(a3: float, a_out: float, repeat: int = 1):
    WDT = BF16 if CFG.get("wdt", "bf16") == "bf16" else FP8
    W2DT = BF16 if CFG.get("w2dt", "fp8") == "bf16" else FP8
    nc = bacc.Bacc("TRN2", debug=False)

    x_d = nc.dram_tensor("x", [C, HW], BF16, kind="ExternalInput")
    w1_d = nc.dram_tensor("w1s", [C, CI], WDT, kind="ExternalInput")
    w2_d = nc.dram_tensor("w2s", [C, 9 * C], W2DT, kind="ExternalInput")
    w3_d = nc.dram_tensor("w3s", [C, C], W2DT, kind="ExternalInput")
    vec_d = nc.dram_tensor("vecs", [C, 4], F32, kind="ExternalInput")
    out_d = nc.dram_tensor("out", [C, HW], BF16, kind="ExternalOutput")

    with TileContext(nc) as tc:
        with (
            tc.tile_pool(name="const", bufs=1) as cpool,
            tc.tile_pool(name="res", bufs=1) as rpool,
            tc.tile_pool(name="g1p", bufs=2) as g1pool,
            tc.tile_pool(name="s3p", bufs=2) as s3pool,
            tc.tile_pool(name="p3p", bufs=3) as p3pool,
            tc.tile_pool(name="ep", bufs=2) as epool,
            tc.tile_pool(name="ps12", bufs=1, space="PSUM") as ps12,
            tc.tile_pool(name="ps3", bufs=2, space="PSUM") as ps3,
        ):
            # ---- constants (scalar DMA queue: keep sync free for x) ----
            w1s = cpool.tile([C, CI], WDT)
            nc.scalar.dma_start(out=w1s, in_=w1_d[:, :])
            w2s = cpool.tile([C, 9 * C], W2DT)
            nc.scalar.dma_start(out=w2s, in_=w2_d[:, :])
            w3s = cpool.tile([C, C], W2DT)
            nc.scalar.dma_start(out=w3s, in_=w3_d[:, :])
            vecs = cpool.tile([C, 4], F32)
            nc.scalar.dma_start(out=vecs, in_=vec_d[:, :])
            t1v = vecs[:, 0:1]
            t2v = vecs[:, 1:2]
            sc3v = vecs[:, 2:3]
            b3v = vecs[:, 3:4]

            # ---- residents ----
            x_sb = rpool.tile([C, HW], BF16)
            xv4 = x_sb.rearrange("p (b r) -> p b r", b=4)       # [128,4,8192]
            ov4 = out_d[:, :].rearrange("p (b r) -> p b r", b=4)
            xv4d = x_d[:, :].rearrange("p (b r) -> p b r", b=4)
            g2b = rpool.tile([128, SLAB * WP], W2DT)            # band slabs
            g2b3 = g2b.rearrange("p (r c) -> p r c", c=WP)

            # s-encoded slab: zero-pad borders (cols 0/257; top rows of
            # bands 0,3; bottom halo rows get real data at b=7)
            nc.vector.memset(g2b3[:, :, 0:1], 0.0)
            nc.vector.memset(g2b3[:, :, WP - 1:WP], 0.0)
            nc.vector.memset(g2b3[0:CI, 0:1, :], 0.0)
            nc.vector.memset(g2b3[96:128, 0:1, :], 0.0)

            def halo(src_g, src_row, dst_g, dst_row):
                nc.sync.dma_start(
                    out=g2b3[CI * dst_g:CI * (dst_g + 1), dst_row:dst_row + 1, :],
                    in_=g2b3[CI * src_g:CI * (src_g + 1), src_row:src_row + 1, :])

            def load(b):
                nc.sync.dma_start(
                    out=xv4[:, :, 1024 * b:1024 * (b + 1)],
                    in_=xv4d[:, :, 1024 * b:1024 * (b + 1)])

            p3ts = {}
            c3s = {}

            def stage1(b):
                dn = 4 * b
                g1t = g1pool.tile([C, BLK], WDT, name="g1t")
                _eng(nc, "g1").tensor_scalar(
                    out=g1t.rearrange("p (b r) -> p b r", b=4),
                    in0=xv4[:, :, 1024 * b:1024 * (b + 1)],
                    scalar1=0.0, scalar2=None, op0=OP.is_gt)
                c1 = ps12.tile([128, 1024], F32, name="c1")
                for g in range(4):
                    for hh in range(2):
                        nc.tensor.matmul(
                            c1[CI * g:CI * (g + 1), 512 * hh:512 * (hh + 1)],
                            w1s,
                            g1t[:, 1024 * g + 512 * hh:
                                1024 * g + 512 * (hh + 1)],
                            start=True, stop=True,
                            tile_position=(0, CI * g),
                        )
                # threshold -> slab rows dn+1..dn+4 (all groups), +-1
                nc.scalar.activation(
                    g2b3[:, dn + 1:dn + 5, 1:W + 1], c1, AF.Sign, bias=t1v)
                if b == 0:
                    halo(2, 1, 1, 0)    # band2 row0 -> band1 top halo
                    halo(1, 1, 2, 0)    # band1 row0 -> band2 top halo
                if b == NBLK - 1:
                    halo(1, 32, 0, 33)  # band1 row31 -> band0 bottom
                    halo(0, 32, 1, 33)  # band0 row31 -> band1 bottom
                    halo(3, 32, 2, 33)  # band3 row31 -> band2 bottom
                    halo(2, 32, 3, 33)  # band2 row31 -> band3 bottom

            def stage2(b):
                """conv2 + th2 + conv3 quarters 0,1 of block b."""
                dn = 4 * b
                c2 = ps12.tile([128, 1024], F32, name="c2")
                # block-diagonal [128,128] weights: one full-array matmul
                # covers all 4 bands per tap (bands are row-aligned).
                for t in range(9):
                    ky, dx = divmod(t, 3)
                    for hh in range(2):
                        nc.tensor.matmul(
                            c2[:, 512 * hh:512 * (hh + 1)],
                            w2s[:, C * t:C * (t + 1)],
                            g2b3[:, dn + ky + 2 * hh:dn + ky + 2 * hh + 2,
                                 dx:dx + W],
                            start=(t == 0), stop=(t == 8),
                        )
                s3g = s3pool.tile([128, 1024], W2DT, name="s3g")
                nc.scalar.activation(s3g, c2, AF.Sign, bias=t2v)
                p3ts[b] = p3pool.tile([128, BLK], BF16, name="p3t")
                c3s[b] = s3g
                conv3q(b, 0)
                conv3q(b, 1)

            def conv3q(b, q):
                s3g = c3s[b]
                c3 = ps3.tile([128, 1024], F32, name="c3")
                for hh in range(2):
                    nc.tensor.matmul(
                        c3[:, 512 * hh:512 * (hh + 1)],
                        w3s[CI * q:CI * (q + 1), :],
                        s3g[CI * q:CI * (q + 1), 512 * hh:512 * (hh + 1)],
                        start=True, stop=True, tile_position=(CI * q, 0),
                    )
                nc.scalar.activation(
                    p3ts[b][:, 1024 * q:1024 * (q + 1)], c3, AF.Prelu,
                    bias=b3v, scale=sc3v, alpha=a3)

            def stage3(b):
                """conv3 quarters 2,3 of block b (ps3 bufs freed by p3 q0,q1)."""
                conv3q(b, 2)
                conv3q(b, 3)

            def epilogue(b):
                p3t = p3ts.pop(b)
                c3s.pop(b)
                xap = xv4[:, :, 1024 * b:1024 * (b + 1)]
                rt = epool.tile([128, BLK], BF16, name="rt")
                _eng(nc, "radd").tensor_tensor(
                    out=rt.rearrange("p (b r) -> p b r", b=4),
                    in0=p3t.rearrange("p (b r) -> p b r", b=4),
                    in1=xap, op=OP.add)
                u = epool.tile([128, BLK], BF16, name="u")
                _eng(nc, "scale").tensor_scalar(
                    out=u, in0=rt, scalar1=a_out, scalar2=None, op0=OP.mult)
                ot = epool.tile([128, BLK], BF16, name="ot")
                _eng(nc, "tmax").tensor_tensor(
                    out=ot, in0=rt, in1=u, op=OP.max)
                nc.sync.dma_start(
                    out=ov4[:, :, 1024 * b:1024 * (b + 1)],
                    in_=ot.rearrange("p (b r) -> p b r", b=4))

            def whole():
                load(0)
                for b in range(NBLK):
                    if b + 1 < NBLK:
                        load(b + 1)
                    stage1(b)
                    if b >= 2:
                        stage3(b - 2)
                    if b >= 1:
                        stage2(b - 1)
                    if b >= 3:
                        epilogue(b - 3)
                # drain, interleaved so V/S/PE overlap through the tail
                stage3(NBLK - 2)
                stage2(NBLK - 1)
                epilogue(NBLK - 3)
                stage3(NBLK - 1)
                epilogue(NBLK - 2)
                epilogue(NBLK - 1)

            if repeat == 1:
                whole()
            else:
                with tc.For_i(0, repeat, 1):
                    whole()

    nc.compile()
    return nc


def _host_params(w1, g1, b1, m1, v1, w2, g2, b2, m2, v2, w3, g3, b3, m3, v3):
    def sgn(w):
        return np.where(w <= 0, -1.0, 1.0)

    w1 = np.asarray(w1, np.float64).reshape(CI, C)
    w2 = np.asarray(w2, np.float64).reshape(CI, CI, 3, 3)
    w3 = np.asarray(w3, np.float64).reshape(C, CI)
    s1, s2, s3 = sgn(w1), sgn(w2), sgn(w3)

    def bnfold(g, b, m, v):
        inv = np.asarray(g, np.float64) / np.sqrt(np.asarray(v, np.float64) + EPS)
        beta = np.asarray(b, np.float64) - np.asarray(m, np.float64) * inv
        return inv, beta

    inv1, beta1 = bnfold(g1, b1, m1, v1)
    inv2, beta2 = bnfold(g2, b2, m2, v2)
    inv3, beta3 = bnfold(g3, b3, m3, v3)

    wdt = (ml_dtypes.bfloat16 if CFG.get("wdt", "bf16") == "bf16"
           else ml_dtypes.float8_e4m3)
    w2dt = (ml_dtypes.bfloat16 if CFG.get("w2dt", "fp8") == "bf16"
            else ml_dtypes.float8_e4m3)
    w1s = (2.0 * s1.T).astype(wdt)                    # [C, CI] lhsT
    # w2s: [128, 9*32]; partitions 32g+c; col block t=(3ky+dx): 2*s2[o,c,ky,dx]
    # groups 1 and 3 process vertically flipped bands -> ky reversed.
    blk = np.zeros((9, CI, CI), np.float64)
    for ky in range(3):
        for dx in range(3):
            blk[3 * ky + dx] = s2[:, :, ky, dx].T         # [c, o]
    blkf = np.zeros_like(blk)
    for ky in range(3):
        for dx in range(3):
            blkf[3 * ky + dx] = blk[3 * (2 - ky) + dx]
    # block-diagonal per tap: [128, 9*128]; bands 1,3 use ky-flipped taps
    w2st = np.zeros((C, 9 * C), np.float64)
    for t in range(9):
        for g in range(4):
            bw = blk[t] if g % 2 == 0 else blkf[t]        # [c, o]
            w2st[CI * g:CI * (g + 1),
                 C * t + CI * g:C * t + CI * (g + 1)] = bw
    w2st = w2st.astype(w2dt)
    w3st = np.tile(s3.T, (4, 1)).astype(w2dt)             # [32g+c, o]

    rs1 = s1.sum(axis=1)

    # Sign activation computes sign(in + bias): bias = -threshold
    t1n = np.tile(-(rs1 - beta1 / inv1), 4).astype(np.float32)
    t2n = np.tile(beta2 / inv2, 4).astype(np.float32)
    sc3 = inv3.astype(np.float32)
    b3f = beta3.astype(np.float32)
    vecs = np.stack([t1n, t2n, sc3, b3f], axis=1)     # [C, 4] f32
    return w1s, w2st, w3st, vecs


def _permute_in(img):
    """[C, H, W] f32 -> [C, HW] bf16, bands 1,3 vertically flipped."""
    xb = img.reshape(C, 4, BH, W).astype(ml_dtypes.bfloat16)
    xb = np.concatenate(
        [xb[:, 0], xb[:, 1, ::-1], xb[:, 2], xb[:, 3, ::-1]], axis=1)
    return np.ascontiguousarray(xb.reshape(C, HW))


def _permute_out(flat):
    """[C, HW] bf16 (flipped-band layout) -> [C, H, W] f32."""
    o = np.asarray(flat).reshape(C, 4, BH, W).astype(np.float32)
    return np.concatenate(
        [o[:, 0], o[:, 1, ::-1], o[:, 2], o[:, 3, ::-1]], axis=1)


last_results = None


def kernel(**inputs):
    global last_results
    x = np.ascontiguousarray(np.asarray(inputs["x"], np.float32))
    w1s, w2st, w3st, vecs = _host_params(
        inputs["w1"], inputs["g1"], inputs["b1"], inputs["m1"], inputs["v1"],
        inputs["w2"], inputs["g2"], inputs["b2"], inputs["m2"], inputs["v2"],
        inputs["w3"], inputs["g3"], inputs["b3"], inputs["m3"], inputs["v3"])
    a3 = float(np.asarray(inputs["a3"]))
    a_out = float(np.asarray(inputs["a_out"]))

    key = (a3, a_out)
    if key not in _CACHE:
        _CACHE[key] = _build(a3, a_out)
    nc = _CACHE[key]

    shared = {"w1s": w1s, "w2s": w2st, "w3s": w3st, "vecs": vecs}
    in_maps = [dict(x=_permute_in(x[b]), **shared) for b in range(B)]
    res = run_bass_kernel_spmd(nc, in_maps, core_ids=list(range(B)))
    last_results = res
    out = np.stack([_permute_out(res.results[b]["out"]) for b in range(B)])
    return out
